# revision 8
# baseline (speedup 1.0000x reference)
import sys, os
sys.path.insert(0, "/opt/trn_rl_repo")
import numpy as np

B, T_FULL, V, E, H = 128, 256, 50000, 256, 512
H2 = H // 2
NTAGS = 76
START_IDX, STOP_IDX = 74, 75
NEG = -10000.0
NPAD = 80           # tags padded to 80
NSUB, NBLK = 8, 10  # 80 = 8*10 ; scores partitions = (s*8 + n_sub)
BL = 16             # sentences per core
NC = 8
BIGNEG = -1.0e9

_cache = {}


def _build(T):
    import concourse.bass as bass
    import concourse.tile as tile
    import concourse.mybir as mybir
    from concourse import bacc
    from concourse.masks import make_identity

    dt = mybir.dt
    f32 = dt.float32
    nc = bacc.Bacc("TRN2", target_bir_lowering=False, debug=False)
    NT = T // 8  # gather calls; rows per call = 128

    # ---------------- DRAM I/O ----------------
    ids_tm = nc.dram_tensor("ids_tm", [128, NT], dt.int32, kind="ExternalInput")
    etab = nc.dram_tensor("etab", [V, E], f32, kind="ExternalInput")
    wcat = nc.dram_tensor("wcat", [2, 128, 4, 4 * H2], f32, kind="ExternalInput")
    wot = nc.dram_tensor("wot", [128, 4, NTAGS], f32, kind="ExternalInput")
    h0t = nc.dram_tensor("h0t", [2, 128, 32], f32, kind="ExternalInput")
    cc0 = nc.dram_tensor("cc0", [2, BL, H2], f32, kind="ExternalInput")
    transrep = nc.dram_tensor("transrep", [128, NBLK * NPAD], f32, kind="ExternalInput")
    ttab = nc.dram_tensor("ttab", [NPAD, NPAD], f32, kind="ExternalInput")
    stoprep = nc.dram_tensor("stoprep", [BL, NPAD], f32, kind="ExternalInput")
    repmat = nc.dram_tensor("repmat", [BL, 128], f32, kind="ExternalInput")
    fvinit = nc.dram_tensor("fvinit", [BL, NPAD], f32, kind="ExternalInput")
    iotaf_in = nc.dram_tensor("iotaf", [BL, NPAD], f32, kind="ExternalInput")

    score_out = nc.dram_tensor("score_out", [BL, 1], f32, kind="ExternalOutput")
    path_out = nc.dram_tensor("path_out", [BL, T], dt.int32, kind="ExternalOutput")

    hhist = nc.dram_tensor("hhist", [2, T, 128, 32], f32)  # DRAM scratch
    pood = nc.dram_tensor("pood", [BL, NPAD], f32)
    feathistd = nc.dram_tensor("feathistd", [BL, T, NPAD], f32)

    AF = mybir.ActivationFunctionType
    OP = mybir.AluOpType

    with tile.TileContext(nc) as tc:
        with tc.tile_pool(name="const", bufs=1) as const, \
             tc.tile_pool(name="work", bufs=3) as work:

            ident = const.tile([128, 128], f32)
            make_identity(nc, ident[:])
            trep = const.tile([128, NBLK, NPAD], f32)
            nc.sync.dma_start(trep[:], transrep[:].rearrange("p (b q) -> p b q", q=NPAD))
            tts = const.tile([NPAD, NPAD], f32)
            nc.sync.dma_start(tts[:], ttab[:])
            srep = const.tile([BL, NPAD], f32)
            nc.sync.dma_start(srep[:], stoprep[:])
            rep = const.tile([BL, 128], f32)
            nc.sync.dma_start(rep[:], repmat[:])
            iotf = const.tile([BL, NPAD], f32)
            nc.sync.dma_start(iotf[:], iotaf_in[:])
            wo = const.tile([128, 4, NTAGS], f32)
            nc.sync.dma_start(wo[:], wot[:])

            # ================= phase LSTM scope =================
            with tc.tile_pool(name="lstm", bufs=1) as lp:
                wct = lp.tile([128, 2, 4, 4 * H2], f32)
                nc.sync.dma_start(wct[:], wcat[:].rearrange("d p k n -> p d k n"))
                eT0 = lp.tile([128, T * BL], f32)
                eT1 = lp.tile([128, T * BL], f32)
                eTs = [eT0, eT1]

                # ---- Phase A: embedding gather (t-major) + transpose ----
                with tc.tile_pool(name="gath", bufs=1) as gp, \
                     tc.tile_pool(name="psA", bufs=2, space="PSUM") as psA:
                    idt = gp.tile([128, NT], dt.int32)
                    nc.sync.dma_start(idt[:], ids_tm[:])
                    embg = gp.tile([128, NT, E], f32)
                    for j in range(NT):
                        nc.gpsimd.indirect_dma_start(
                            out=embg[:, j, :], out_offset=None, in_=etab[:],
                            in_offset=bass.IndirectOffsetOnAxis(ap=idt[:, j:j + 1], axis=0))
                    for j in range(NT):
                        for kh in range(2):
                            tp = psA.tile([128, 128], f32, tag="tpose")
                            nc.tensor.transpose(tp[:], embg[:, j, kh * 128:(kh + 1) * 128],
                                                ident[:])
                            nc.scalar.copy(eTs[kh][:, j * 128:(j + 1) * 128], tp[:])

                # ---- Phase B: BiLSTM ----
                with tc.tile_pool(name="psB", bufs=1, space="PSUM") as psB, \
                     tc.tile_pool(name="psBt", bufs=2, space="PSUM") as psBt:
                    hT = [work.tile([128, 32], f32, tag=f"hT{d}", name=f"hT{d}") for d in range(2)]
                    cc = [work.tile([BL, H2], f32, tag=f"cc{d}", name=f"cc{d}") for d in range(2)]
                    for d in range(2):
                        nc.sync.dma_start(hT[d][:], h0t[d])
                        nc.sync.dma_start(cc[d][:], cc0[d])

                    for t in range(T):
                        for d in range(2):
                            tx = t if d == 0 else (T - 1 - t)
                            g = psB.tile([BL, 4 * H2], f32, tag=f"g{d}")
                            for nh in range(2):
                                for kt in range(4):
                                    if kt < 2:
                                        lhs = hT[d][:, kt * 16:(kt + 1) * 16]
                                    else:
                                        lhs = eTs[kt - 2][:, tx * BL:(tx + 1) * BL]
                                    nc.tensor.matmul(
                                        g[:, nh * 512:(nh + 1) * 512],
                                        lhs, wct[:, d, kt, nh * 512:(nh + 1) * 512],
                                        start=(kt == 0), stop=(kt == 3))
                            th = work.tile([BL, 4 * H2], f32, tag=f"th{d}")
                            nc.scalar.activation(th[:], g[:], AF.Tanh, scale=0.5)
                            u = work.tile([BL, H2], f32, tag=f"u{d}")
                            nc.vector.scalar_tensor_tensor(u[:], th[:, 256:512], 1.0,
                                                           cc[d][:], op0=OP.add, op1=OP.mult)
                            r = work.tile([BL, H2], f32, tag=f"r{d}")
                            nc.vector.scalar_tensor_tensor(r[:], th[:, 0:256], 1.0,
                                                           th[:, 512:768], op0=OP.add,
                                                           op1=OP.mult)
                            nc.vector.scalar_tensor_tensor(cc[d][:], u[:], 0.5, r[:],
                                                           op0=OP.mult, op1=OP.add)
                            thc = work.tile([BL, H2], f32, tag=f"thc{d}")
                            nc.scalar.activation(thc[:], cc[d][:], AF.Tanh, scale=0.5)
                            hh = work.tile([BL, H2], f32, tag=f"hh{d}")
                            nc.vector.scalar_tensor_tensor(hh[:], th[:, 768:1024], 1.0,
                                                           thc[:], op0=OP.add, op1=OP.mult)
                            for kh in range(2):
                                tp = psBt.tile([128, BL], f32, tag="tpose2")
                                nc.tensor.transpose(tp[:], hh[:, kh * 128:(kh + 1) * 128],
                                                    ident[0:BL, 0:BL])
                                nc.scalar.copy(hT[d][:, kh * 16:(kh + 1) * 16], tp[:])
                            nc.sync.dma_start(hhist[d, tx], hT[d][:])

            # ================= late scope =================
            with tc.tile_pool(name="late", bufs=1) as late:
                fvh = late.tile([BL, (T + 1) * NPAD], f32)
                patht = late.tile([BL, T], dt.int32)
                zpad = late.tile([BL, T * 4], f32)
                nc.gpsimd.memset(zpad[:], 0.0)
                nc.sync.dma_start(feathistd[:, :, NTAGS + 0:NPAD],
                                  zpad[:].rearrange("s (t q) -> s t q", q=4))

                # ---- Phase C: feats ----
                with tc.tile_pool(name="fTp", bufs=1) as fTp, \
                     tc.tile_pool(name="psC", bufs=2, space="PSUM") as psC:
                    fT = fTp.tile([NTAGS, T * BL], f32)
                    NCH = (T * BL) // 512
                    for ch in range(NCH):
                        fp = psC.tile([NTAGS, 512], f32, tag="fps")
                        for kt in range(4):
                            kd, kh = divmod(kt, 2)
                            rhsb = work.tile([128, 32, BL], f32, tag="hrd")
                            nc.sync.dma_start(
                                rhsb[:],
                                hhist[kd, ch * 32:(ch + 1) * 32, :, kh * 16:(kh + 1) * 16]
                                .rearrange("t p s -> p t s"))
                            nc.tensor.matmul(fp[:], wo[:, kt, :],
                                             rhsb[:].rearrange("p t s -> p (t s)"),
                                             start=(kt == 0), stop=(kt == 3))
                        nc.vector.tensor_copy(fT[:, ch * 512:(ch + 1) * 512], fp[:])
                    for t2 in range(T):
                        nc.sync.dma_start(
                            feathistd[:, t2, 0:NTAGS].rearrange("s n -> n s"),
                            fT[:, t2 * BL:(t2 + 1) * BL])

                # ---- Phase D: viterbi forward ----
                with tc.tile_pool(name="psD", bufs=2, space="PSUM") as psD:
                    nc.sync.dma_start(fvh[:, 0:NPAD], fvinit[:])
                    for t in range(T):
                        fve = psD.tile([128, NPAD], f32, tag="fve")
                        nc.tensor.matmul(fve[:], rep[:], fvh[:, t * NPAD:(t + 1) * NPAD],
                                         start=True, stop=True)
                        sco = work.tile([128, NBLK, NPAD], f32, tag="sco")
                        nc.vector.tensor_tensor(
                            sco[:], trep[:],
                            fve[:].rearrange("p (o q) -> p o q", o=1).to_broadcast([128, NBLK, NPAD]),
                            op=OP.add)
                        poo = work.tile([128, NBLK], f32, tag="poo")
                        nc.vector.tensor_reduce(poo[:], sco[:], axis=mybir.AxisListType.X,
                                                op=OP.max)
                        nc.sync.dma_start(
                            pood[:].rearrange("s (u b) -> (s u) b", u=NSUB), poo[:])
                        ut = work.tile([BL, NPAD], f32, tag="ut")
                        nc.sync.dma_start(ut[:], pood[:])
                        fh = work.tile([BL, NPAD], f32, tag="fh")
                        nc.sync.dma_start(fh[:], feathistd[:, t, :])
                        nc.vector.tensor_tensor(fvh[:, (t + 1) * NPAD:(t + 2) * NPAD],
                                                ut[:], fh[:], op=OP.add)

                # ---- Phase E: terminal + backtrack ----
                with tc.tile_pool(name="psE", bufs=2, space="PSUM") as psE:
                    vt = work.tile([BL, NPAD], f32, tag="vt")
                    nc.vector.tensor_tensor(vt[:], fvh[:, T * NPAD:(T + 1) * NPAD],
                                            srep[:], op=OP.add)
                    mx = work.tile([BL, 8], f32, tag="mx")
                    nc.vector.max(mx[:], vt[:])
                    yi = work.tile([BL, 8], dt.uint32, tag="yi")
                    nc.vector.max_index(yi[:], mx[:], vt[:])
                    nc.vector.tensor_copy(patht[:, T - 1:T], yi[:, 0:1])
                    sc0 = work.tile([BL, 1], f32, tag="sc0")
                    nc.vector.tensor_copy(sc0[:], mx[:, 0:1])
                    nc.sync.dma_start(score_out[:], sc0[:])

                    for t in range(T - 2, -1, -1):
                        yvf = work.tile([BL, 1], f32, tag="yvf")
                        nc.vector.tensor_copy(yvf[:], yi[:, 0:1])
                        oh = work.tile([BL, NPAD], f32, tag="oh")
                        nc.vector.tensor_scalar(oh[:], iotf[:], yvf[:], None,
                                                op0=OP.is_equal)
                        ohp = psE.tile([NPAD, BL], f32, tag="ohp")
                        nc.tensor.transpose(ohp[:], oh[:], ident[0:BL, 0:BL])
                        ohs = work.tile([NPAD, BL], f32, tag="ohs")
                        nc.scalar.copy(ohs[:], ohp[:])
                        tsel = psE.tile([BL, NPAD], f32, tag="tsel")
                        nc.tensor.matmul(tsel[:], ohs[:], tts[:], start=True, stop=True)
                        v = work.tile([BL, NPAD], f32, tag="v")
                        nc.vector.tensor_tensor(v[:], fvh[:, (t + 1) * NPAD:(t + 2) * NPAD],
                                                tsel[:], op=OP.add)
                        mx = work.tile([BL, 8], f32, tag="mx")
                        nc.vector.max(mx[:], v[:])
                        yi = work.tile([BL, 8], dt.uint32, tag="yi")
                        nc.vector.max_index(yi[:], mx[:], v[:])
                        nc.vector.tensor_copy(patht[:, t:t + 1], yi[:, 0:1])

                    nc.sync.dma_start(path_out[:], patht[:])

    nc.compile()
    return nc


def _host_prep(T, input_ids, embed_table, Wih_f, Whh_f, bih_f, bhh_f,
               Wih_b, Whh_b, bih_b, bhh_b, h0, c0, W_out, b_out, transitions):
    assert np.abs(bih_f).max() == 0 and np.abs(bhh_f).max() == 0
    assert np.abs(bih_b).max() == 0 and np.abs(bhh_b).max() == 0
    assert np.abs(b_out).max() == 0

    f = np.float32

    def mk_wcat(Wih, Whh):
        w = np.concatenate([0.5 * Whh.T, Wih.T], axis=0).astype(f).copy()  # [512, 1024]
        w[:, 2 * H2:3 * H2] *= 2.0
        return w.reshape(4, 128, 4 * H2).transpose(1, 0, 2)  # [128, 4, 1024]
    wcat = np.stack([mk_wcat(Wih_f, Whh_f), mk_wcat(Wih_b, Whh_b)]).astype(f)

    wot = np.ascontiguousarray((0.5 * W_out.T).astype(f).reshape(4, 128, NTAGS)
                               .transpose(1, 0, 2))

    tt = np.full((NPAD, NPAD), BIGNEG, f)
    tt[:NTAGS, :NTAGS] = transitions
    trep = np.zeros((128, NBLK * NPAD), f)
    for m in range(128):
        ns = m % NSUB
        trep[m] = tt[ns * NBLK:(ns + 1) * NBLK, :].reshape(-1)
    srep = np.broadcast_to(tt[STOP_IDX], (BL, NPAD)).astype(f).copy()
    repmat = np.zeros((BL, 128), f)
    for m in range(128):
        repmat[m // NSUB, m] = 1.0
    fvinit = np.full((BL, NPAD), NEG, f)
    fvinit[:, START_IDX] = 0.0
    iotaf = np.broadcast_to(np.arange(NPAD, dtype=f), (BL, NPAD)).copy()

    ins = []
    for c in range(NC):
        sl = slice(c * BL, (c + 1) * BL)
        ids = np.asarray(input_ids[sl])
        g = ids.T.reshape(-1).astype(np.int32)          # t-major [T*16]
        ids_tm = np.ascontiguousarray(g.reshape(T * BL // 128, 128).T)
        h0c = np.asarray(h0[:, sl], dtype=f)
        h0t = np.zeros((2, 128, 32), f)
        for d in range(2):
            ht = (2.0 * h0c[d]).T                        # [256, 16]
            h0t[d] = ht.reshape(2, 128, BL).transpose(1, 0, 2).reshape(128, 32)
        cc0 = np.ascontiguousarray(2.0 * np.asarray(c0[:, sl], dtype=f))
        ins.append(dict(ids_tm=ids_tm, etab=np.asarray(embed_table, dtype=f),
                        wcat=wcat, wot=wot, h0t=h0t, cc0=cc0, transrep=trep, ttab=tt,
                        stoprep=srep, repmat=repmat, fvinit=fvinit, iotaf=iotaf))
    return ins


def kernel(**inputs):
    from concourse.bass_utils import run_bass_kernel_spmd
    inputs = {k: np.asarray(v) for k, v in inputs.items()}
    T = inputs["input_ids"].shape[1]
    if T not in _cache:
        _cache[T] = _build(T)
    nc = _cache[T]
    ins = _host_prep(T, **inputs)
    res = run_bass_kernel_spmd(nc, ins, list(range(NC))).results
    score = np.concatenate([r["score_out"][:, 0] for r in res]).astype(np.float32)
    path = np.concatenate([r["path_out"] for r in res]).astype(np.int32)
    return score, path
